# revision 2
# baseline (speedup 1.0000x reference)
"""Block-sparse flash attention (Phi-3-small pattern) on 8 Trainium2 cores, v2.

Strip-major schedule, all-fp16, tensor-parallel over heads (core r owns
heads [4r, 4r+4), all sharing GQA KV head r).

Geometry per head (c = (7-h) % 8):
  block (qb, kb) active iff qb >= kb and (qb-kb < 16 or kb % 8 == c)
  - LOCAL: k-tile kt (128 k rows = blocks {2kt, 2kt+1}) covers q span
    [128kt, 128kt+1088): causal triangle at the diagonal 128 cols, and a
    masked 64x64 corner at cols [128kt+1024, +1088) rows 0:64 (kt<=7).
  - TAIL: vertical blocks {c, c+8} gathered into one 128-row k-tile,
    active q in [1024+64c, 2048); row-block masks are a host-built
    per-head multiplier (tmz).

Schedule: outer loop over 8 q-strips of 256 columns.  Within a strip,
for each active k-tile: one QK matmul per head-PAIR (2 heads x 256 q =
512 moving cols; kT weights shared across all 4 heads), one 4-head exp
on ACT, masks on GpSimd, then (lagged by `lag` pieces) PV matmuls into
per-strip PSUM banks (2 heads per bank, V weights shared) plus a 4-head
fp16 rowsum accumulation on DVE.  Strip epilogue: tail PV, PSUM->DRAM
DMA of the unnormalized out^T, and SBUF->DRAM DMA of the fp16 rowsum
accumulator.  The 128-row reduction of the accumulator, normalization,
and final transpose happen on the host.

Compared to the kt-major baseline this removes the third PE stream
(rowsum ones-matmuls), all on-chip transposes and the DVE epilogue, and
cuts PE/ACT instruction counts roughly 3x (512-col moving operands,
1024-col exps).
"""

import sys

import numpy as np

for _p in ("/opt/trn_rl_repo", "/root/.axon_site/_ro/trn_rl_repo"):
    if _p not in sys.path:
        sys.path.append(_p)

from contextlib import ExitStack

import concourse.bacc as bacc
import concourse.mybir as mybir
import concourse.tile as tile
from concourse.bass_utils import run_bass_kernel_spmd


S = 2048
D = 128
H = 32
HKV = 8
NCORES = 8
NH = H // NCORES          # 4 heads per core
SCALE = 0.08838834764831845
NKT = 16                  # 128-row k-tiles
NSTRIP = 8                # 256-col q strips
STRIP = 256
SPAN = 1088

F16 = mybir.dt.float16
F32 = mybir.dt.float32
NPF16 = np.float16


def _pieces(s):
    """Active (kt, a, b) local pieces for strip s, ascending kt."""
    lo, hi = STRIP * s, STRIP * s + STRIP
    out = []
    for kt in range(NKT):
        a = max(128 * kt, lo)
        b = min(128 * kt + SPAN, S, hi)
        if a < b:
            out.append((kt, a, b))
    return out


def build_program(lag=5, eTb=9, owb=2, scb=3, tail_after=3):
    nc = bacc.Bacc("TRN2", target_bir_lowering=False, debug=False)
    qs_d = nc.dram_tensor("qs", [NSTRIP, 128, NH, STRIP], F16,
                          kind="ExternalInput").ap()
    kT_d = nc.dram_tensor("kT", [128, S], F16, kind="ExternalInput").ap()
    vR_d = nc.dram_tensor("vR", [128, S], F16, kind="ExternalInput").ap()
    kvT_d = nc.dram_tensor("kvT", [128, NH, 128], F16,
                           kind="ExternalInput").ap()
    vv_d = nc.dram_tensor("vv", [128, NH, 128], F16,
                          kind="ExternalInput").ap()
    tri_d = nc.dram_tensor("tri", [128, NH, 128], F16,
                           kind="ExternalInput").ap()
    stepb_d = nc.dram_tensor("stepb", [2, NH, 1024], F16,
                             kind="ExternalInput").ap()
    tsel_d = nc.dram_tensor("tailsel", [2, 128], F16,
                            kind="ExternalInput").ap()
    outT = nc.dram_tensor("outT", [2, 128, 2, S], F16,
                          kind="ExternalOutput").ap()
    accd = nc.dram_tensor("accd", [NSTRIP, 128, NH, STRIP], F16,
                          kind="ExternalOutput").ap()

    Exp = mybir.ActivationFunctionType.Exp

    with tile.TileContext(nc) as tc, ExitStack() as ctx:
        const = ctx.enter_context(tc.tile_pool(name="const", bufs=1))
        qsp = ctx.enter_context(tc.tile_pool(name="qs", bufs=NSTRIP))
        eTp = ctx.enter_context(tc.tile_pool(name="eTl", bufs=eTb))
        etp2 = ctx.enter_context(tc.tile_pool(name="ett", bufs=NH))
        accp = ctx.enter_context(tc.tile_pool(name="acc", bufs=2))
        osbp = ctx.enter_context(tc.tile_pool(name="osb", bufs=4))
        scp = ctx.enter_context(tc.tile_pool(name="sc", bufs=scb, space="PSUM"))
        owp = ctx.enter_context(tc.tile_pool(name="ow", bufs=owb,
                                             space="PSUM"))

        # DMA order matters: strip 0 must be able to start ASAP, so its
        # operands (qs[0], kT) go first, then what the early pipeline
        # needs (vR for PV at lag, tri for the first diag), then the rest.
        qs_sb = [qsp.tile([128, NH, STRIP], F16, tag="qs", name=f"qs{s}")
                 for s in range(NSTRIP)]
        kT_sb = const.tile([128, S], F16, tag="kT")
        vR_sb = const.tile([128, S], F16, tag="vR")
        kvT_sb = const.tile([128, NH, 128], F16, tag="kvT")
        vv_sb = const.tile([128, NH, 128], F16, tag="vv")
        tri_sb = const.tile([128, NH, 128], F16, tag="tri")
        stepb_sb = const.tile([2, NH, 1024], F16, tag="stepb")
        tailsel_sb = const.tile([2, 128], F16, tag="tailsel")

        # Spread the startup loads over four engines' DMA queues so
        # strip 0's operands land in parallel instead of serializing.
        # Priority-ordered, chunked input loads.  Strip order below is
        # 1..7 then 0, so strip 1's operands come first.  kT/vR stream in
        # strip-sized chunks on the scalar queue while qs strips stream
        # on the sync queue; first chunks are small so compute starts
        # within ~3us.
        nc.sync.dma_start(qs_sb[1][:, 0:2, :], qs_d[1][:, 0:2, :])
        nc.sync.dma_start(kT_sb[:, 0:512], kT_d[:, 0:512])
        nc.sync.dma_start(qs_sb[1][:, 2:4, :], qs_d[1][:, 2:4, :])
        nc.sync.dma_start(vR_sb[:, 0:512], vR_d[:, 0:512])
        nc.sync.dma_start(qs_sb[2][:], qs_d[2])
        nc.sync.dma_start(tri_sb[:], tri_d[:])
        nc.sync.dma_start(kT_sb[:, 512:1280], kT_d[:, 512:1280])
        nc.sync.dma_start(qs_sb[3][:], qs_d[3])
        nc.sync.dma_start(stepb_sb[:], stepb_d[:])
        nc.sync.dma_start(tailsel_sb[:], tsel_d[:])
        nc.sync.dma_start(kvT_sb[:], kvT_d[:])
        nc.sync.dma_start(vR_sb[:, 512:1280], vR_d[:, 512:1280])
        nc.sync.dma_start(qs_sb[4][:], qs_d[4])
        nc.sync.dma_start(kT_sb[:, 1280:2048], kT_d[:, 1280:2048])
        nc.sync.dma_start(vv_sb[:], vv_d[:])
        nc.sync.dma_start(qs_sb[5][:], qs_d[5])
        nc.sync.dma_start(vR_sb[:, 1280:2048], vR_d[:, 1280:2048])
        nc.sync.dma_start(qs_sb[6][:], qs_d[6])
        nc.sync.dma_start(qs_sb[7][:], qs_d[7])
        nc.sync.dma_start(qs_sb[0][:], qs_d[0])

        pending = []

        def flush(force=False):
            while pending and (force or len(pending) > lag):
                pending.pop(0)()

        eT_tail = [None] * NH

        def emit_tails():
            # Tail QK + exp + row-mask for all 4 heads (q in [1024, 2048)).
            for h in range(NH):
                sc = scp.tile([128, NH, STRIP], F32, tag="sc",
                              name=f"tsc{h}")
                scf = sc.rearrange("p a b -> p (a b)")
                for x in (0, 512):
                    for j in (x // 256, x // 256 + 1):
                        nc.tensor.matmul(scf[:, 256 * j:256 * j + 256],
                                         kvT_sb[:, h, :],
                                         qs_sb[4 + j][:, h, :],
                                         start=j % 2 == 0, stop=False)
                    # additive -30000 row-block mask (kills exp to 0)
                    nc.tensor.matmul(scf[:, x:x + 512], tailsel_sb[:],
                                     stepb_sb[:, h, x:x + 512],
                                     start=False, stop=True)
                et = etp2.tile([128, 1024], F16, tag="ett", name=f"ett{h}")
                nc.scalar.activation(et[:], scf[:], Exp, scale=SCALE)
                eT_tail[h] = et

        tails_done = False

        # Big strips in the middle, tiny strip 0 last: minimizes both the
        # startup DMA gating (strip 1's data loads fastest) and the final
        # drain (strip 0's epilogue is the shortest).
        for pos, s in enumerate([1, 2, 3, 4, 5, 6, 7, 0]):
            s_lo, s_hi = STRIP * s, STRIP * s + STRIP
            pieces = _pieces(s)

            acc = accp.tile([128, NH, STRIP], F16, tag="acc",
                            name=f"acc{s}")
            nc.gpsimd.memset(acc[:], 0.0)
            acc_flat = acc.rearrange("p h j -> p (h j)")

            # One PV matmul per piece.  start=True ONLY on the bank's
            # first matmul: the PE zeroes the entire PSUM zero-region on
            # start, so a second start into the same bank would wipe the
            # accumulation (and the first start covers columns the first
            # piece doesn't write).
            plan = [(kt, a, b, i == 0) for i, (kt, a, b) in
                    enumerate(pieces)]
            n_parts = len(plan)
            total_mm = {0: n_parts + (2 if s >= 4 else 0),
                        1: n_parts + (2 if s >= 4 else 0)}

            ow = [owp.tile([128, 2, STRIP], F32, tag="ow",
                           name=f"ow{s}_{p}") for p in range(2)]
            seen = {0: 0, 1: 0}

            for (kt, a, b, fresh) in plan:
                w = b - a
                sc = scp.tile([128, NH, STRIP], F32, tag="sc",
                              name=f"sc{s}_{kt}")
                for p in (0, 1):
                    nc.tensor.matmul(sc[:, 2 * p:2 * p + 2, 0:w],
                                     kT_sb[:, 128 * kt:128 * kt + 128],
                                     qs_sb[s][:, 2 * p:2 * p + 2,
                                              a - s_lo:b - s_lo],
                                     start=True, stop=True)
                eT = eTp.tile([128, NH, STRIP], F16, tag="eT",
                              name=f"eT{s}_{kt}")
                nc.scalar.activation(eT[:, :, 0:w], sc[:, :, 0:w], Exp,
                                     scale=SCALE)
                if kt in (2 * s, 2 * s + 1):
                    # causal triangle (piece starts exactly at 128*kt)
                    nc.gpsimd.tensor_mul(eT[:, :, 0:128], eT[:, :, 0:128],
                                         tri_sb[:])
                if kt <= 7 and a <= 128 * kt + 1024 < b:
                    rel = 128 * kt + 1024 - a
                    nc.gpsimd.memset(eT[0:64, :, rel:rel + 64], 0.0)
                def stage_pv(kt=kt, a=a, b=b, w=w, eT=eT, fresh=fresh,
                             ow=ow, seen=seen, total_mm=total_mm,
                             s_lo=s_lo, acc=acc, acc_flat=acc_flat):
                    for p in (0, 1):
                        seen[p] += 1
                        nc.tensor.matmul(
                            ow[p][:, :, a - s_lo:b - s_lo],
                            vR_sb[:, 128 * kt:128 * kt + 128],
                            eT[:, 2 * p:2 * p + 2, 0:b - a],
                            start=fresh,
                            stop=seen[p] == total_mm[p])
                    # rowsum accumulation rides in the lagged stage so
                    # strip-epilogue copies reach the DVE queue before the
                    # next strip's adds
                    if w == STRIP:
                        eT_flat = eT.rearrange("p h j -> p (h j)")
                        nc.vector.tensor_add(acc_flat[:], acc_flat[:],
                                             eT_flat[:])
                    else:
                        nc.vector.tensor_add(acc[:, :, a - s_lo:b - s_lo],
                                             acc[:, :, a - s_lo:b - s_lo],
                                             eT[:, :, 0:w])

                flush()
                pending.append(stage_pv)

            def epilogue(s=s, s_lo=s_lo, s_hi=s_hi, acc=acc, ow=ow,
                         seen=seen, total_mm=total_mm, last=pos == 7):
                if s >= 4:
                    for h in range(NH):
                        p = h // 2
                        seen[p] += 1
                        nc.tensor.matmul(
                            ow[p][:, h % 2, :],
                            vv_sb[:, h, :],
                            eT_tail[h][:, s_lo - 1024:s_hi - 1024],
                            start=False,
                            stop=seen[p] == total_mm[p],
                            skip_group_check=True)
                        nc.vector.tensor_add(
                            acc[:, h, :], acc[:, h, :],
                            eT_tail[h][:, s_lo - 1024:s_hi - 1024])
                nc.scalar.dma_start(accd[s], acc[:])
                for p in (0, 1):
                    osb = osbp.tile([128, 2, STRIP], F16, tag="osb",
                                    name=f"osb{s}_{p}")
                    nc.vector.tensor_copy(osb[:], ow[p][:])
                    eng2 = (nc.scalar if p == 0 else nc.sync) if last \
                        else nc.sync
                    eng2.dma_start(outT[p][:, :, s_lo:s_hi], osb[:])

            pending.append(epilogue)

            if not tails_done and pos == tail_after:
                emit_tails()
                tails_done = True

        flush(force=True)
    nc.compile()
    return nc


def make_core_inputs(query, key, value, core):
    q3 = query.reshape(S, H, D)
    k3 = key.reshape(S, HKV, D)
    v3 = value.reshape(S, HKV, D)
    r = core
    K = k3[:, r, :]
    V = v3[:, r, :]

    qs = np.empty((NSTRIP, 128, NH, STRIP), NPF16)
    kvT = np.empty((128, NH, 128), NPF16)
    vv = np.empty((128, NH, 128), NPF16)
    tri = np.empty((128, NH, 128), NPF16)
    stepb = np.empty((2, NH, 1024), NPF16)

    kk = np.arange(128)[:, None]
    qq2 = np.arange(128)[None, :]
    tri_pat = (qq2 >= kk).astype(NPF16)
    jj = np.arange(1024)

    for hl in range(NH):
        hg = NH * r + hl
        c = (7 - hg) % 8
        qh = q3[:, hg, :]                      # [S, 128]
        for s in range(NSTRIP):
            qs[s, :, hl, :] = qh[STRIP * s:STRIP * s + STRIP, :].T
        kvT[:, hl, 0:64] = K[64 * c:64 * c + 64, :].T
        kvT[:, hl, 64:128] = K[64 * (c + 8):64 * (c + 8) + 64, :].T
        vv[0:64, hl, :] = V[64 * c:64 * c + 64, :]
        vv[64:128, hl, :] = V[64 * (c + 8):64 * (c + 8) + 64, :]
        tri[:, hl, :] = tri_pat
        stepb[0, hl, :] = np.where(jj >= 64 * c, 0, -30000).astype(NPF16)
        stepb[1, hl, :] = np.where(jj >= 512 + 64 * c, 0,
                                   -30000).astype(NPF16)

    vRe = np.ascontiguousarray(
        V.reshape(NKT, 128, D).transpose(1, 0, 2).reshape(128, S))

    return {
        "qs": qs,
        "kT": np.ascontiguousarray(K.T).astype(NPF16),
        "vR": vRe.astype(NPF16),
        "kvT": kvT,
        "vv": vv,
        "tri": tri,
        "stepb": stepb,
        "tailsel": np.concatenate(
            [np.r_[np.ones(64), np.zeros(64)][None, :],
             np.r_[np.zeros(64), np.ones(64)][None, :]]).astype(NPF16),
    }


_PROGRAM = None


def _get_program():
    global _PROGRAM
    if _PROGRAM is None:
        _PROGRAM = build_program()
    return _PROGRAM


def run(query, key, value, trace=False):
    nc = _get_program()
    in_maps = [make_core_inputs(query, key, value, r) for r in range(NCORES)]
    br = run_bass_kernel_spmd(nc, in_maps, list(range(NCORES)), trace=trace)
    out = np.empty((S, H * D), np.float32)
    for r in range(NCORES):
        outT = br.results[r]["outT"]           # [2, 128, 2, S] f16
        accd = br.results[r]["accd"]           # [NSTRIP, 128, NH, STRIP] f16
        dq = accd.astype(np.float32).sum(axis=1)      # [NSTRIP, NH, STRIP]
        dq = dq.transpose(1, 0, 2).reshape(NH, S)     # [hl, q]
        for hl in range(NH):
            hg = NH * r + hl
            o = outT[hl // 2][:, hl % 2, :].astype(np.float32)
            out[:, D * hg:D * hg + D] = (o / dq[hl][None, :]).T
    return out, br


def kernel(query, key, value):
    out, _ = run(np.asarray(query), np.asarray(key), np.asarray(value))
    return out
